# revision 23
# baseline (speedup 1.0000x reference)
"""Gaussian KDE (brute-force, bandwidth^2 = 1) on 8 Trainium2 NeuronCores.

Math:
    out_i = log( sum_j w_j * exp(-||x_i - y_j||^2/2) ) - (d/2) log(2pi) - log(sum_j w_j)
          = log( sum_j exp(x_i . y_j + b_j) ) - ||x_i||^2/2 - consts
    with b_j = log(w_j) - ||y_j||^2/2.

Queries sharded 8 ways (512/core, 4 PSUM-partition tiles). Per core:
    - scores: K=35 bf16 matmuls, stationary = query tile [35, 128], moving =
      train slices. Operands are pre-scaled so PSUM holds C1*s + C2b/32
      (C1 = 4/ln2, C2b the bf16 Schraudolph bias): x rows = C1*x dims plus
      three 4.0 rows; y rows = y dims + (C1/4)*b hi + lo + C2b/128 row.
      K=35 <= 64, so consecutive matmuls alternate PE row groups via
      tile_position (0,0)/(64,0) and run pairwise-concurrently.
    - exp+sum, two unit types balanced by a credit scheduler across engines:
      * ACT unit [128, 1536] (3 banks): table-exp in place with
        scale=1/C1, bias=-C2b/(32*C1), free-dim sum fused via accum_out.
      * DVE unit [128, 512] (1 bank): tensor_scalar (mult 32, max 0) whose
        int16 result IS the bf16 bit pattern of exp (Schraudolph; the max
        makes int16 wrap impossible), then tensor_reduce of the bitcast.
    - final: per query tile reduce partials, ln, subtract per-query const.
"""

import numpy as np
import ml_dtypes

_Q, _N, _D = 4096, 65536, 32
_NCORES = 8
_QSHARD = _Q // _NCORES          # 512 queries per core
_K = 34                          # 32 dims + bias hi/lo (incl C2 const)
_QT = 4                          # query tiles per core

_BF16 = ml_dtypes.bfloat16

_C1 = 4.0 / float(np.log(2.0))


def _c2b():
    f = (np.arange(100000, dtype=np.float64) + 0.5) / 100000.0
    m0 = np.mean((1.0 + f) * 2.0 ** (-f))
    m1 = np.mean(2.0 ** (-f))
    delta = (m0 - 1.0) / m1
    return float(127 * 128 - delta * 128)


_C2B = _c2b()

# e5m2 Schraudolph bias for the flipped fp8 stream
def _c2e5():
    f = (np.arange(100000, dtype=np.float64) + 0.5) / 100000.0
    m0 = np.mean((1.0 + f) * 2.0 ** (-f))
    m1 = np.mean(2.0 ** (-f))
    delta = (m0 - 1.0) / m1
    return float(15 * 4.0 - delta * 4.0)


_C2E5 = _c2e5()

# per-query shift estimate for the fp8 window (added back exactly at the end)
_A_FIT = 4.42465707
_C_FIT = -17.07362259
_AOFF = 8.0 + 1.0 - float(np.log(57344.0))

# flipped-stream supertiles (256 trains each) handled by DVE + PE fp8-DR
_NFLIP = 42                      # 42 * 256 = 10752 trains
_QT_TRAINS = 65536 - _NFLIP * 256   # 54784 per query tile via [q,t] units
_ND = 26                         # 26 * 512 = 13312 trains via qt-DVE units
_NA_TRAINS = _QT_TRAINS - _ND * 512  # 41472 via ACT units (1536/1024 alt)
_ACT_NS = 1790.0
_DVE_NS = 1347.0
_FLIP_NS = 1192.0

_prog_cache: dict = {}


def _unit_schedule():
    """Credit-scheduled unit type sequence (shared by all 4 query tiles)."""
    seq = []
    na, nd, ta, td = 0, 0, 0.0, 0.0
    while na < _NA or nd < _ND:
        if nd >= _ND or (na < _NA and ta <= td):
            seq.append('A')
            na += 1
            ta += _ACT_NS
        else:
            seq.append('D')
            nd += 1
            td += _DVE_NS
    return seq


def _build_program(n_trains: int):
    import concourse.bass as bass
    import concourse.tile as tile
    from concourse import bacc, mybir

    f32 = mybir.dt.float32
    bf16 = mybir.dt.bfloat16
    i16 = mybir.dt.int16
    i8 = mybir.dt.int8
    f8 = mybir.dt.float8e5
    DR = mybir.MatmulPerfMode.DoubleRow

    nc = bacc.Bacc("TRN2", target_bir_lowering=False, debug=False,
                   num_devices=_NCORES)

    KY = _K + 1   # y rows incl the flip ones-row
    y_d = nc.dram_tensor("yext", [KY, n_trains], bf16, kind="ExternalInput")
    x_d = nc.dram_tensor("xext", [KY, _QSHARD], bf16, kind="ExternalInput")
    on_d = nc.dram_tensor("ones8", [128, 256], i8, kind="ExternalInput")
    out_d = nc.dram_tensor("out", [128, _QT], f32, kind="ExternalOutput")
    of_d = nc.dram_tensor("outf", [1, _QSHARD], f32, kind="ExternalOutput")

    with tile.TileContext(nc) as tc:
        with (
            tc.tile_pool(name="const", bufs=1) as cpool,
            tc.tile_pool(name="q16", bufs=6) as qpool,
            tc.tile_pool(name="small", bufs=2) as spool,
            tc.tile_pool(name="ps", bufs=1, space="PSUM") as ppool,
        ):
            xsb = cpool.tile([128, _QSHARD], bf16)
            nc.sync.dma_start(xsb[0:KY, :], x_d[:])
            nc.sync.dma_start(xsb[64:64 + KY, :], x_d[:])
            ones_dr = cpool.tile([128, 2, 128], f8)
            nc.sync.dma_start(ones_dr[:].bitcast(i8), on_d[:])
            bias_sb = cpool.tile([128, 1], f32)
            nc.vector.memset(bias_sb[:], -_C2B / (32.0 * _C1))

            # y resident in SBUF, both row-group strips, 8 DMA pieces each
            ysb = cpool.tile([128, n_trains], bf16)
            npc = n_trains // 8
            for p in range(8):
                nc.sync.dma_start(ysb[0:KY, p * npc:(p + 1) * npc],
                                  y_d[:, p * npc:(p + 1) * npc])
                nc.sync.dma_start(ysb[64:64 + KY, p * npc:(p + 1) * npc],
                                  y_d[:, p * npc:(p + 1) * npc])

            sall = cpool.tile([128, 4 * (41 + _ND)], f32)

            # PSUM: A units alternate [0:1536],[1536:2560]; D/flip share
            # [2560:3072],[3072:3584]; flip accumulator at [3584:4096]
            ps = ppool.tile([128, 7 * 512], f32)
            acc = ppool.tile([128, _QSHARD], f32)

            rg_par = [0]

            def score_mm(qt, dst, t0, width):
                for j in range(width // 512):
                    rg = 64 * (rg_par[0] & 1)
                    rg_par[0] += 1
                    nc.tensor.matmul(
                        out=ps[:, dst + j * 512: dst + (j + 1) * 512],
                        lhsT=xsb[rg:rg + _K, qt * 128:(qt + 1) * 128],
                        rhs=ysb[rg:rg + _K, t0 + j * 512: t0 + (j + 1) * 512],
                        start=True, stop=True,
                        tile_position=(rg, 0),
                    )

            NCQ = 41 + _ND          # partial columns reserved per query tile
            agen = [0]
            dgen = [0]
            pcol = [0] * _QT
            cur = [_NFLIP * 256] * _QT
            arem = [_NA_TRAINS] * _QT
            drem = [_ND] * _QT

            pend = {}
            first_red = [True]
            fемit = [0]

            def emit_reduce(s, last=False):
                rhs = pend.pop(s)
                nc.tensor.matmul(out=acc[:], lhsT=ones_dr[:], rhs=rhs,
                                 start=first_red[0], stop=last,
                                 perf_mode=DR, skip_group_check=True)
                first_red[0] = False

            def do_a_step(step):
                for i in range(_QT):
                    qt = (step + i) % _QT
                    if arem[qt] <= 0:
                        continue
                    slotw = 1536 if (agen[0] & 1) == 0 else 1024
                    dst = 0 if (agen[0] & 1) == 0 else 1536
                    agen[0] += 1
                    w = min(slotw, arem[qt])
                    w -= w % 512
                    score_mm(qt, dst, cur[qt], w)
                    c = qt * NCQ + pcol[qt]
                    nc.scalar.activation(
                        ps[:, dst:dst + w], ps[:, dst:dst + w],
                        mybir.ActivationFunctionType.Exp,
                        bias=bias_sb[:], scale=1.0 / _C1,
                        accum_out=sall[:, c:c + 1])
                    pcol[qt] += 1
                    cur[qt] += w
                    arem[qt] -= w

            def do_d_step(step):
                for i in range(_QT):
                    qt = (step + i) % _QT
                    if drem[qt] <= 0:
                        continue
                    dst = 2560 + 512 * (dgen[0] & 1)
                    dgen[0] += 1
                    score_mm(qt, dst, cur[qt], 512)
                    q16 = qpool.tile([128, 512], i16)
                    nc.vector.tensor_scalar(
                        q16[:], ps[:, dst:dst + 512], 32.0, 0.0,
                        mybir.AluOpType.mult, mybir.AluOpType.max)
                    c = qt * NCQ + pcol[qt]
                    nc.vector.tensor_reduce(
                        sall[:, c:c + 1], q16[:].bitcast(bf16),
                        axis=mybir.AxisListType.X, op=mybir.AluOpType.add)
                    pcol[qt] += 1
                    cur[qt] += 512
                    drem[qt] -= 1

            def do_flip_super():
                s = fемit[0]
                fемit[0] += 1
                t0 = s * 256
                dst = 2560
                for h in range(2):
                    rg = 64 * (rg_par[0] & 1)
                    rg_par[0] += 1
                    nc.tensor.matmul(
                        out=ps[:, dst + h * 512: dst + (h + 1) * 512],
                        lhsT=ysb[rg:rg + _K + 1,
                                 t0 + h * 128: t0 + (h + 1) * 128],
                        rhs=xsb[rg:rg + _K + 1, :],
                        start=True, stop=True,
                        tile_position=(rg, 0),
                    )
                ex = qpool.tile([128, 2, _QSHARD], i8)
                nc.vector.tensor_scalar(
                    ex[:], ps[:, dst:dst + 1024], 1.0, 0.0,
                    mybir.AluOpType.mult, mybir.AluOpType.max)
                pend[s] = ex[:].bitcast(f8)
                if s >= 3:
                    emit_reduce(s - 3)

            ta, td = 0.0, 0.0
            step = 0
            est_steps = 33 + _ND
            while (max(arem) > 0 or max(drem) > 0 or fемit[0] < _NFLIP):
                if (ta <= td or max(drem) <= 0) and max(arem) > 0:
                    do_a_step(step)
                    ta += 4 * _ACT_NS
                elif max(drem) > 0:
                    do_d_step(step)
                    td += 4 * _DVE_NS
                # pace flip supers uniformly so their single-buffered PSUM
                # region is always free by the time the next one issues
                while (fемit[0] < _NFLIP
                       and fемit[0] <= step * _NFLIP // est_steps):
                    do_flip_super()
                    td += _FLIP_NS
                step += 1
            for s in range(max(0, _NFLIP - 3), _NFLIP):
                emit_reduce(s, last=(s == _NFLIP - 1))

            # qt-side partial sums per query tile (no ln - host finishes)
            fin = spool.tile([128, _QT], f32)
            for qt in range(_QT):
                nc.vector.tensor_reduce(
                    fin[:, qt:qt + 1],
                    sall[:, qt * NCQ:qt * NCQ + pcol[qt]],
                    axis=mybir.AxisListType.X, op=mybir.AluOpType.add)
            nc.sync.dma_start(out_d[:], fin[:])
            # flip totals (row 0 of acc)
            frow = spool.tile([1, _QSHARD], f32)
            nc.scalar.copy(frow[:], acc[0:1, :])
            nc.sync.dma_start(of_d[:], frow[:])

    nc.compile()
    return nc


def _get_program(n_trains: int):
    if n_trains not in _prog_cache:
        _prog_cache[n_trains] = _build_program(n_trains)
    return _prog_cache[n_trains]


def _prep_inputs(X, X_train, sample_weight):
    X = np.ascontiguousarray(np.asarray(X, dtype=np.float32))
    Y = np.ascontiguousarray(np.asarray(X_train, dtype=np.float32))
    w = np.ascontiguousarray(np.asarray(sample_weight, dtype=np.float32))
    n = Y.shape[0]

    w64 = w.astype(np.float64)
    b64 = np.log(np.maximum(w64, 1e-300)) - 0.5 * np.sum(
        Y.astype(np.float64) ** 2, axis=1)
    b64 = np.clip(b64, -35.0, None)
    cb64 = (_C1 * b64 + _C2B / 32.0) / 4.0
    bhi = cb64.astype(np.float32).astype(_BF16)
    blo = (cb64 - bhi.astype(np.float64)).astype(np.float32).astype(_BF16)

    yext = np.empty((_K + 1, n), dtype=_BF16)
    yext[0:32] = Y.astype(_BF16).T
    yext[32] = bhi
    yext[33] = blo
    yext[34] = np.ones(n, dtype=_BF16)   # flip ones-row

    const = 0.5 * _D * np.log(2.0 * np.pi) + np.log(np.sum(w64))
    xsq = np.sum(X.astype(np.float64) ** 2, axis=1)
    r = np.sqrt(xsq)
    m_est = _A_FIT * r + _C_FIT
    # flip shift row: psum_flip = C1*s + C2B/32 + row34; want C1*(s-m-AOFF)+C2E5
    row34_t = -_C1 * (m_est + _AOFF) + _C2E5 - _C2B / 32.0
    row34 = row34_t.astype(np.float32).astype(_BF16)
    # effective applied shift, recovered exactly from the rounded row
    m_eff = -(row34.astype(np.float64) - _C2E5 + _C2B / 32.0) / _C1 - _AOFF
    fscale = np.exp(m_eff + _AOFF)                  # flip sum multiplier
    dv_all = 0.5 * xsq + const

    in_maps = []
    extras = []
    for c in range(_NCORES):
        sl = slice(c * _QSHARD, (c + 1) * _QSHARD)
        xq = X[sl]
        xext = np.empty((_K + 1, _QSHARD), dtype=_BF16)
        xext[0:32] = (_C1 * xq.astype(np.float64)).astype(_BF16).T
        xext[32] = np.full(_QSHARD, 4.0, dtype=_BF16)
        xext[33] = np.full(_QSHARD, 4.0, dtype=_BF16)
        xext[34] = row34[sl]
        ones8 = np.full((128, 256), 0x3c, dtype=np.int8)  # e5m2 1.0
        in_maps.append({"yext": yext, "xext": xext, "ones8": ones8})
        extras.append((fscale[sl], dv_all[sl]))
    return in_maps, extras


def _gather(results, extras):
    out = np.empty(_Q, dtype=np.float32)
    for c in range(_NCORES):
        qt_part = results[c]["out"].T.reshape(_QSHARD).astype(np.float64)
        flip = results[c]["outf"].reshape(_QSHARD).astype(np.float64)
        fscale, dv = extras[c]
        total = qt_part + fscale * flip
        out[c * _QSHARD:(c + 1) * _QSHARD] = (np.log(total) - dv)
    return out


def kernel(X, X_train, sample_weight, _want_timing=False):
    from concourse.bass_utils import run_bass_kernel_spmd

    nc = _get_program(_N)
    in_maps, extras = _prep_inputs(X, X_train, sample_weight)
    kres = run_bass_kernel_spmd(
        nc, in_maps, core_ids=list(range(_NCORES)),
        trace=bool(_want_timing),
    )
    out = _gather(kres.results, extras)
    if _want_timing:
        return out, kres
    return out


# revision 24
# speedup vs baseline: 2.3580x; 2.3580x over previous
"""Gaussian KDE (brute-force, bandwidth^2 = 1) on 8 Trainium2 NeuronCores.

Math:
    out_i = log( sum_j w_j * exp(-||x_i - y_j||^2/2) ) - (d/2) log(2pi) - log(sum_j w_j)
          = log( sum_j exp(x_i . y_j + b_j) ) - ||x_i||^2/2 - consts
    with b_j = log(w_j) - ||y_j||^2/2.

Queries sharded 8 ways (512/core, 4 PSUM-partition tiles). Per core:
    - scores: K=35 bf16 matmuls, stationary = query tile [35, 128], moving =
      train slices. Operands are pre-scaled so PSUM holds C1*s + C2b/32
      (C1 = 4/ln2, C2b the bf16 Schraudolph bias): x rows = C1*x dims plus
      three 4.0 rows; y rows = y dims + (C1/4)*b hi + lo + C2b/128 row.
      K=35 <= 64, so consecutive matmuls alternate PE row groups via
      tile_position (0,0)/(64,0) and run pairwise-concurrently.
    - exp+sum, two unit types balanced by a credit scheduler across engines:
      * ACT unit [128, 1536] (3 banks): table-exp in place with
        scale=1/C1, bias=-C2b/(32*C1), free-dim sum fused via accum_out.
      * DVE unit [128, 512] (1 bank): tensor_scalar (mult 32, max 0) whose
        int16 result IS the bf16 bit pattern of exp (Schraudolph; the max
        makes int16 wrap impossible), then tensor_reduce of the bitcast.
    - final: per query tile reduce partials, ln, subtract per-query const.
"""

import numpy as np
import ml_dtypes

_Q, _N, _D = 4096, 65536, 32
_NCORES = 8
_QSHARD = _Q // _NCORES          # 512 queries per core
_K = 34                          # 32 dims + bias hi/lo (incl C2 const)
_QT = 4                          # query tiles per core

_BF16 = ml_dtypes.bfloat16

_C1 = 4.0 / float(np.log(2.0))


def _c2b():
    f = (np.arange(100000, dtype=np.float64) + 0.5) / 100000.0
    m0 = np.mean((1.0 + f) * 2.0 ** (-f))
    m1 = np.mean(2.0 ** (-f))
    delta = (m0 - 1.0) / m1
    return float(127 * 128 - delta * 128)


_C2B = _c2b()

# per query tile: trains covered by ACT units (1536 each) and DVE units (512)
_NA = 29                         # 29 * 1536 = 44544
_ND = 41                         # 41 * 512  = 20992 ; total 65536
_ACT_NS = 1724.0                 # (1536+192)/1.2 + 284
_DVE_NS = 1252.0                 # (120+512)/0.96 + (58+512)/0.96

_prog_cache: dict = {}


def _unit_schedule():
    """Credit-scheduled unit type sequence (shared by all 4 query tiles)."""
    seq = []
    na, nd, ta, td = 0, 0, 0.0, 0.0
    while na < _NA or nd < _ND:
        if nd >= _ND or (na < _NA and ta <= td):
            seq.append('A')
            na += 1
            ta += _ACT_NS
        else:
            seq.append('D')
            nd += 1
            td += _DVE_NS
    return seq


def _build_program(n_trains: int):
    import concourse.bass as bass
    import concourse.tile as tile
    from concourse import bacc, mybir

    f32 = mybir.dt.float32
    bf16 = mybir.dt.bfloat16
    i16 = mybir.dt.int16

    nc = bacc.Bacc("TRN2", target_bir_lowering=False, debug=False,
                   num_devices=_NCORES)

    y_d = nc.dram_tensor("yext", [_K, n_trains], bf16, kind="ExternalInput")
    x_d = nc.dram_tensor("xext", [_K, _QSHARD], bf16, kind="ExternalInput")
    dv_d = nc.dram_tensor("dv", [128, _QT], f32, kind="ExternalInput")
    out_d = nc.dram_tensor("out", [128, _QT], f32, kind="ExternalOutput")

    seq = _unit_schedule()

    with tile.TileContext(nc) as tc:
        with (
            tc.tile_pool(name="const", bufs=1) as cpool,
            tc.tile_pool(name="q16", bufs=6) as qpool,
            tc.tile_pool(name="small", bufs=2) as spool,
            tc.tile_pool(name="ps", bufs=1, space="PSUM") as ppool,
        ):
            xsb = cpool.tile([128, _QSHARD], bf16)
            nc.sync.dma_start(xsb[0:_K, :], x_d[:])
            nc.sync.dma_start(xsb[64:64 + _K, :], x_d[:])
            dv_sb = cpool.tile([128, _QT], f32)
            nc.sync.dma_start(dv_sb[:], dv_d[:])
            bias_sb = cpool.tile([128, 1], f32)
            nc.vector.memset(bias_sb[:], -_C2B / (32.0 * _C1))

            # y resident in SBUF, both row-group strips, 8 DMA pieces each
            ysb = cpool.tile([128, n_trains], bf16)
            npc = n_trains // 8
            for p in range(8):
                nc.sync.dma_start(ysb[0:_K, p * npc:(p + 1) * npc],
                                  y_d[:, p * npc:(p + 1) * npc])
                nc.sync.dma_start(ysb[64:64 + _K, p * npc:(p + 1) * npc],
                                  y_d[:, p * npc:(p + 1) * npc])

            sall = cpool.tile([128, len(seq) * _QT], f32)

            # PSUM: A units double-buffered at [0:1536],[1536:3072];
            # D units at [3072:3584],[3584:4096]
            ps = ppool.tile([128, 8 * 512], f32)

            rg_par = [0]

            def score_mm(qt, dst, t0, width):
                for j in range(width // 512):
                    rg = 64 * (rg_par[0] & 1)
                    rg_par[0] += 1
                    nc.tensor.matmul(
                        out=ps[:, dst + j * 512: dst + (j + 1) * 512],
                        lhsT=xsb[rg:rg + _K, qt * 128:(qt + 1) * 128],
                        rhs=ysb[rg:rg + _K, t0 + j * 512: t0 + (j + 1) * 512],
                        start=True, stop=True,
                        tile_position=(rg, 0),
                    )

            col = [0]
            gen = {'A': 0, 'D': 0}
            cur = [0] * _QT
            for typ in seq:
                for qt in range(_QT):
                    t0 = cur[qt]
                    if typ == 'A':
                        dst = 1536 * (gen['A'] & 1)
                        gen['A'] += 1
                        score_mm(qt, dst, t0, 1536)
                        nc.scalar.activation(
                            ps[:, dst:dst + 1536], ps[:, dst:dst + 1536],
                            mybir.ActivationFunctionType.Exp,
                            bias=bias_sb[:], scale=1.0 / _C1,
                            accum_out=sall[:, col[0]:col[0] + 1])
                        cur[qt] = t0 + 1536
                    else:
                        dst = 3072 + 512 * (gen['D'] & 1)
                        gen['D'] += 1
                        score_mm(qt, dst, t0, 512)
                        q16 = qpool.tile([128, 512], i16)
                        nc.vector.tensor_scalar(
                            q16[:], ps[:, dst:dst + 512], 32.0, 0.0,
                            mybir.AluOpType.mult, mybir.AluOpType.max)
                        nc.vector.tensor_reduce(
                            sall[:, col[0]:col[0] + 1], q16[:].bitcast(bf16),
                            axis=mybir.AxisListType.X, op=mybir.AluOpType.add)
                        cur[qt] = t0 + 512
                    col[0] += 1

            nun = len(seq)
            fin = spool.tile([128, _QT], f32)
            for qt in range(_QT):
                red = spool.tile([128, 1], f32)
                nc.vector.tensor_reduce(
                    red[:], sall[:, qt:qt + 4 * (nun - 1) + 1:4],
                    axis=mybir.AxisListType.X, op=mybir.AluOpType.add)
                lg = spool.tile([128, 1], f32)
                nc.scalar.activation(lg[:], red[:],
                                     mybir.ActivationFunctionType.Ln)
                nc.vector.tensor_sub(fin[:, qt:qt + 1], lg[:],
                                     dv_sb[:, qt:qt + 1])
            nc.sync.dma_start(out_d[:], fin[:])

    nc.compile()
    return nc


def _get_program(n_trains: int):
    if n_trains not in _prog_cache:
        _prog_cache[n_trains] = _build_program(n_trains)
    return _prog_cache[n_trains]


def _prep_inputs(X, X_train, sample_weight):
    X = np.ascontiguousarray(np.asarray(X, dtype=np.float32))
    Y = np.ascontiguousarray(np.asarray(X_train, dtype=np.float32))
    w = np.ascontiguousarray(np.asarray(sample_weight, dtype=np.float32))
    n = Y.shape[0]

    w64 = w.astype(np.float64)
    b64 = np.log(np.maximum(w64, 1e-300)) - 0.5 * np.sum(
        Y.astype(np.float64) ** 2, axis=1)
    b64 = np.clip(b64, -35.0, None)
    cb64 = (_C1 * b64 + _C2B / 32.0) / 4.0
    bhi = cb64.astype(np.float32).astype(_BF16)
    blo = (cb64 - bhi.astype(np.float64)).astype(np.float32).astype(_BF16)

    yext = np.empty((_K, n), dtype=_BF16)
    yext[0:32] = Y.astype(_BF16).T
    yext[32] = bhi
    yext[33] = blo

    const = 0.5 * _D * np.log(2.0 * np.pi) + np.log(np.sum(w64))
    xsq = np.sum(X.astype(np.float64) ** 2, axis=1)
    dv_all = (0.5 * xsq + const).astype(np.float32)

    in_maps = []
    for c in range(_NCORES):
        sl = slice(c * _QSHARD, (c + 1) * _QSHARD)
        xq = X[sl]
        xext = np.empty((_K, _QSHARD), dtype=_BF16)
        xext[0:32] = (_C1 * xq.astype(np.float64)).astype(_BF16).T
        xext[32] = np.full(_QSHARD, 4.0, dtype=_BF16)
        xext[33] = np.full(_QSHARD, 4.0, dtype=_BF16)
        dv = np.ascontiguousarray(dv_all[sl].reshape(_QT, 128).T)
        in_maps.append({"yext": yext, "xext": xext, "dv": dv})
    return in_maps


def _gather(results):
    out = np.empty(_Q, dtype=np.float32)
    for c in range(_NCORES):
        res = results[c]["out"]                        # [128, QT]
        out[c * _QSHARD:(c + 1) * _QSHARD] = res.T.reshape(_QSHARD)
    return out


def kernel(X, X_train, sample_weight, _want_timing=False):
    from concourse.bass_utils import run_bass_kernel_spmd

    nc = _get_program(_N)
    in_maps = _prep_inputs(X, X_train, sample_weight)
    kres = run_bass_kernel_spmd(
        nc, in_maps, core_ids=list(range(_NCORES)),
        trace=bool(_want_timing),
    )
    out = _gather(kres.results)
    if _want_timing:
        return out, kres
    return out


# revision 25
# speedup vs baseline: 2.3700x; 1.0051x over previous
"""Gaussian KDE (brute-force, bandwidth^2 = 1) on 8 Trainium2 NeuronCores.

Math:
    out_i = log( sum_j w_j * exp(-||x_i - y_j||^2/2) ) - (d/2) log(2pi) - log(sum_j w_j)
          = log( sum_j exp(x_i . y_j + b_j) ) - ||x_i||^2/2 - consts
    with b_j = log(w_j) - ||y_j||^2/2.

Queries sharded 8 ways (512/core, 4 PSUM-partition tiles). Per core:
    - scores: K=35 bf16 matmuls, stationary = query tile [35, 128], moving =
      train slices. Operands are pre-scaled so PSUM holds C1*s + C2b/32
      (C1 = 4/ln2, C2b the bf16 Schraudolph bias): x rows = C1*x dims plus
      three 4.0 rows; y rows = y dims + (C1/4)*b hi + lo + C2b/128 row.
      K=35 <= 64, so consecutive matmuls alternate PE row groups via
      tile_position (0,0)/(64,0) and run pairwise-concurrently.
    - exp+sum, two unit types balanced by a credit scheduler across engines:
      * ACT unit [128, 1536] (3 banks): table-exp in place with
        scale=1/C1, bias=-C2b/(32*C1), free-dim sum fused via accum_out.
      * DVE unit [128, 512] (1 bank): tensor_scalar (mult 32, max 0) whose
        int16 result IS the bf16 bit pattern of exp (Schraudolph; the max
        makes int16 wrap impossible), then tensor_reduce of the bitcast.
    - final: per query tile reduce partials, ln, subtract per-query const.
"""

import numpy as np
import ml_dtypes

_Q, _N, _D = 4096, 65536, 32
_NCORES = 8
_QSHARD = _Q // _NCORES          # 512 queries per core
_K = 34                          # 32 dims + bias hi/lo (incl C2 const)
_QT = 4                          # query tiles per core

_BF16 = ml_dtypes.bfloat16

_C1 = 4.0 / float(np.log(2.0))


def _c2b():
    f = (np.arange(100000, dtype=np.float64) + 0.5) / 100000.0
    m0 = np.mean((1.0 + f) * 2.0 ** (-f))
    m1 = np.mean(2.0 ** (-f))
    delta = (m0 - 1.0) / m1
    return float(127 * 128 - delta * 128)


_C2B = _c2b()

# per query tile: trains covered by ACT units (1536 each) and DVE units (512)
_NA = 29                         # 29 * 1536 = 44544
_ND = 41                         # 41 * 512  = 20992 ; total 65536
_ACT_NS = 1724.0                 # (1536+192)/1.2 + 284
_DVE_NS = 1252.0                 # (120+512)/0.96 + (58+512)/0.96

_prog_cache: dict = {}


def _unit_schedule():
    """Credit-scheduled unit type sequence (shared by all 4 query tiles)."""
    seq = []
    na, nd, ta, td = 0, 0, 0.0, 0.0
    while na < _NA or nd < _ND:
        if nd >= _ND or (na < _NA and ta <= td):
            seq.append('A')
            na += 1
            ta += _ACT_NS
        else:
            seq.append('D')
            nd += 1
            td += _DVE_NS
    return seq


def _build_program(n_trains: int):
    import concourse.bass as bass
    import concourse.tile as tile
    from concourse import bacc, mybir

    f32 = mybir.dt.float32
    bf16 = mybir.dt.bfloat16
    i16 = mybir.dt.int16

    nc = bacc.Bacc("TRN2", target_bir_lowering=False, debug=False,
                   num_devices=_NCORES)

    y_d = nc.dram_tensor("yext", [_K, n_trains], bf16, kind="ExternalInput")
    x_d = nc.dram_tensor("xext", [_K, _QSHARD], bf16, kind="ExternalInput")
    out_d = nc.dram_tensor("out", [128, _QT], f32, kind="ExternalOutput")

    seq = _unit_schedule()

    with tile.TileContext(nc) as tc:
        with (
            tc.tile_pool(name="const", bufs=1) as cpool,
            tc.tile_pool(name="q16", bufs=6) as qpool,
            tc.tile_pool(name="small", bufs=2) as spool,
            tc.tile_pool(name="ps", bufs=1, space="PSUM") as ppool,
        ):
            xsb = cpool.tile([128, _QSHARD], bf16)
            nc.sync.dma_start(xsb[0:_K, :], x_d[:])
            nc.sync.dma_start(xsb[64:64 + _K, :], x_d[:])
            bias_sb = cpool.tile([128, 1], f32)
            nc.vector.memset(bias_sb[:], -_C2B / (32.0 * _C1))

            # y resident in SBUF, both row-group strips, 8 DMA pieces each
            ysb = cpool.tile([128, n_trains], bf16)
            pieces = [2048, 2048, 4096] + [8192] * 7
            off = 0
            for w in pieces:
                nc.sync.dma_start(ysb[0:_K, off:off + w],
                                  y_d[:, off:off + w])
                nc.sync.dma_start(ysb[64:64 + _K, off:off + w],
                                  y_d[:, off:off + w])
                off += w

            NCQ = _NA + (_ND + 1) // 2
            sall = cpool.tile([128, NCQ * _QT], f32)

            # PSUM: A units double-buffered at [0:1536],[1536:3072];
            # D units at [3072:3584],[3584:4096]
            ps = ppool.tile([128, 8 * 512], f32)

            rg_par = [0]

            def score_mm(qt, dst, t0, width):
                for j in range(width // 512):
                    rg = 64 * (rg_par[0] & 1)
                    rg_par[0] += 1
                    nc.tensor.matmul(
                        out=ps[:, dst + j * 512: dst + (j + 1) * 512],
                        lhsT=xsb[rg:rg + _K, qt * 128:(qt + 1) * 128],
                        rhs=ysb[rg:rg + _K, t0 + j * 512: t0 + (j + 1) * 512],
                        start=True, stop=True,
                        tile_position=(rg, 0),
                    )

            pcol = [0] * _QT
            gen = {'A': 0, 'D': 0}
            cur = [0] * _QT
            half = [None] * _QT       # pending first half of a D pair
            for typ in seq:
                for qt in range(_QT):
                    t0 = cur[qt]
                    if typ == 'A':
                        dst = 1536 * (gen['A'] & 1)
                        gen['A'] += 1
                        score_mm(qt, dst, t0, 1536)
                        c = qt * NCQ + pcol[qt]
                        pcol[qt] += 1
                        nc.scalar.activation(
                            ps[:, dst:dst + 1536], ps[:, dst:dst + 1536],
                            mybir.ActivationFunctionType.Exp,
                            bias=bias_sb[:], scale=1.0 / _C1,
                            accum_out=sall[:, c:c + 1])
                        cur[qt] = t0 + 1536
                    else:
                        dst = 3072 + 512 * (gen['D'] & 1)
                        gen['D'] += 1
                        score_mm(qt, dst, t0, 512)
                        if half[qt] is None:
                            q16 = qpool.tile([128, 2, 512], i16)
                            nc.vector.tensor_scalar(
                                q16[:, 0, :], ps[:, dst:dst + 512], 32.0, 0.0,
                                mybir.AluOpType.mult, mybir.AluOpType.max)
                            half[qt] = q16
                        else:
                            q16 = half[qt]
                            half[qt] = None
                            nc.vector.tensor_scalar(
                                q16[:, 1, :], ps[:, dst:dst + 512], 32.0, 0.0,
                                mybir.AluOpType.mult, mybir.AluOpType.max)
                            c = qt * NCQ + pcol[qt]
                            pcol[qt] += 1
                            nc.vector.tensor_reduce(
                                sall[:, c:c + 1], q16[:].bitcast(bf16),
                                axis=mybir.AxisListType.XY,
                                op=mybir.AluOpType.add)
                        cur[qt] = t0 + 512

            for qt in range(_QT):
                if half[qt] is not None:
                    q16 = half[qt]
                    c = qt * NCQ + pcol[qt]
                    pcol[qt] += 1
                    nc.vector.tensor_reduce(
                        sall[:, c:c + 1], q16[:, 0, :].bitcast(bf16),
                        axis=mybir.AxisListType.X, op=mybir.AluOpType.add)

            fin = spool.tile([128, _QT], f32)
            for qt in range(_QT):
                nc.vector.tensor_reduce(
                    fin[:, qt:qt + 1], sall[:, qt * NCQ:qt * NCQ + pcol[qt]],
                    axis=mybir.AxisListType.X, op=mybir.AluOpType.add)
            nc.sync.dma_start(out_d[:], fin[:])

    nc.compile()
    return nc


def _get_program(n_trains: int):
    if n_trains not in _prog_cache:
        _prog_cache[n_trains] = _build_program(n_trains)
    return _prog_cache[n_trains]


def _prep_inputs(X, X_train, sample_weight):
    X = np.ascontiguousarray(np.asarray(X, dtype=np.float32))
    Y = np.ascontiguousarray(np.asarray(X_train, dtype=np.float32))
    w = np.ascontiguousarray(np.asarray(sample_weight, dtype=np.float32))
    n = Y.shape[0]

    w64 = w.astype(np.float64)
    b64 = np.log(np.maximum(w64, 1e-300)) - 0.5 * np.sum(
        Y.astype(np.float64) ** 2, axis=1)
    b64 = np.clip(b64, -35.0, None)
    cb64 = (_C1 * b64 + _C2B / 32.0) / 4.0
    bhi = cb64.astype(np.float32).astype(_BF16)
    blo = (cb64 - bhi.astype(np.float64)).astype(np.float32).astype(_BF16)

    yext = np.empty((_K, n), dtype=_BF16)
    yext[0:32] = Y.astype(_BF16).T
    yext[32] = bhi
    yext[33] = blo

    const = 0.5 * _D * np.log(2.0 * np.pi) + np.log(np.sum(w64))
    xsq = np.sum(X.astype(np.float64) ** 2, axis=1)
    dv_all = (0.5 * xsq + const).astype(np.float32)

    in_maps = []
    dvs = []
    for c in range(_NCORES):
        sl = slice(c * _QSHARD, (c + 1) * _QSHARD)
        xq = X[sl]
        xext = np.empty((_K, _QSHARD), dtype=_BF16)
        xext[0:32] = (_C1 * xq.astype(np.float64)).astype(_BF16).T
        xext[32] = np.full(_QSHARD, 4.0, dtype=_BF16)
        xext[33] = np.full(_QSHARD, 4.0, dtype=_BF16)
        dv = np.ascontiguousarray(dv_all[sl].reshape(_QT, 128).T)
        in_maps.append({"yext": yext, "xext": xext})
        dvs.append(dv_all[sl].astype(np.float64))
    return in_maps, dvs


def _gather(results, dvs):
    out = np.empty(_Q, dtype=np.float32)
    for c in range(_NCORES):
        tot = results[c]["out"].T.reshape(_QSHARD).astype(np.float64)
        out[c * _QSHARD:(c + 1) * _QSHARD] = np.log(tot) - dvs[c]
    return out


def kernel(X, X_train, sample_weight, _want_timing=False):
    from concourse.bass_utils import run_bass_kernel_spmd

    nc = _get_program(_N)
    in_maps, dvs = _prep_inputs(X, X_train, sample_weight)
    kres = run_bass_kernel_spmd(
        nc, in_maps, core_ids=list(range(_NCORES)),
        trace=bool(_want_timing),
    )
    out = _gather(kres.results, dvs)
    if _want_timing:
        return out, kres
    return out


# revision 26
# speedup vs baseline: 2.3854x; 1.0065x over previous
"""Gaussian KDE (brute-force, bandwidth^2 = 1) on 8 Trainium2 NeuronCores.

Math:
    out_i = log( sum_j w_j * exp(-||x_i - y_j||^2/2) ) - (d/2) log(2pi) - log(sum_j w_j)
          = log( sum_j exp(x_i . y_j + b_j) ) - ||x_i||^2/2 - consts
    with b_j = log(w_j) - ||y_j||^2/2.

Queries sharded 8 ways (512/core, 4 PSUM-partition tiles). Per core:
    - scores: K=35 bf16 matmuls, stationary = query tile [35, 128], moving =
      train slices. Operands are pre-scaled so PSUM holds C1*s + C2b/32
      (C1 = 4/ln2, C2b the bf16 Schraudolph bias): x rows = C1*x dims plus
      three 4.0 rows; y rows = y dims + (C1/4)*b hi + lo + C2b/128 row.
      K=35 <= 64, so consecutive matmuls alternate PE row groups via
      tile_position (0,0)/(64,0) and run pairwise-concurrently.
    - exp+sum, two unit types balanced by a credit scheduler across engines:
      * ACT unit [128, 1536] (3 banks): table-exp in place with
        scale=1/C1, bias=-C2b/(32*C1), free-dim sum fused via accum_out.
      * DVE unit [128, 512] (1 bank): tensor_scalar (mult 32, max 0) whose
        int16 result IS the bf16 bit pattern of exp (Schraudolph; the max
        makes int16 wrap impossible), then tensor_reduce of the bitcast.
    - final: per query tile reduce partials, ln, subtract per-query const.
"""

import numpy as np
import ml_dtypes

_Q, _N, _D = 4096, 65536, 32
_NCORES = 8
_QSHARD = _Q // _NCORES          # 512 queries per core
_K = 34                          # 32 dims + bias hi/lo (incl C2 const)
_QT = 4                          # query tiles per core

_BF16 = ml_dtypes.bfloat16

_C1 = 4.0 / float(np.log(2.0))


def _c2b():
    f = (np.arange(100000, dtype=np.float64) + 0.5) / 100000.0
    m0 = np.mean((1.0 + f) * 2.0 ** (-f))
    m1 = np.mean(2.0 ** (-f))
    delta = (m0 - 1.0) / m1
    return float(127 * 128 - delta * 128)


_C2B = _c2b()

# per query tile: trains covered by ACT units (1536 each) and DVE units (512)
_NA = 29                         # 29 * 1536 = 44544
_ND = 41                         # 41 * 512  = 20992 ; total 65536
_ACT_NS = 1724.0                 # (1536+192)/1.2 + 284
_DVE_NS = 1252.0                 # (120+512)/0.96 + (58+512)/0.96

_prog_cache: dict = {}


def _unit_schedule():
    """Credit-scheduled unit type sequence (shared by all 4 query tiles)."""
    seq = []
    na, nd, ta, td = 0, 0, 0.0, -5000.0   # start with D units: less y needed
    while na < _NA or nd < _ND:
        if nd >= _ND or (na < _NA and ta <= td):
            seq.append('A')
            na += 1
            ta += _ACT_NS
        else:
            seq.append('D')
            nd += 1
            td += _DVE_NS
    return seq


def _build_program(n_trains: int):
    import concourse.bass as bass
    import concourse.tile as tile
    from concourse import bacc, mybir

    f32 = mybir.dt.float32
    bf16 = mybir.dt.bfloat16
    i16 = mybir.dt.int16

    nc = bacc.Bacc("TRN2", target_bir_lowering=False, debug=False,
                   num_devices=_NCORES)

    y_d = nc.dram_tensor("yext", [_K, n_trains], bf16, kind="ExternalInput")
    x_d = nc.dram_tensor("xext", [_K, _QSHARD], bf16, kind="ExternalInput")
    out_d = nc.dram_tensor("out", [128, _QT], f32, kind="ExternalOutput")

    seq = _unit_schedule()

    with tile.TileContext(nc) as tc:
        with (
            tc.tile_pool(name="const", bufs=1) as cpool,
            tc.tile_pool(name="q16", bufs=8) as qpool,
            tc.tile_pool(name="small", bufs=2) as spool,
            tc.tile_pool(name="ps", bufs=1, space="PSUM") as ppool,
        ):
            xsb = cpool.tile([128, _QSHARD], bf16)
            nc.sync.dma_start(xsb[0:_K, :], x_d[:])
            nc.sync.dma_start(xsb[64:64 + _K, :], x_d[:])
            bias_sb = cpool.tile([128, 1], f32)
            nc.vector.memset(bias_sb[:], -_C2B / (32.0 * _C1))

            # y resident in SBUF, both row-group strips, 8 DMA pieces each
            ysb = cpool.tile([128, n_trains], bf16)
            pieces = [1024, 1024, 2048, 4096] + [8192] * 7
            off = 0
            for w in pieces:
                nc.sync.dma_start(ysb[0:_K, off:off + w],
                                  y_d[:, off:off + w])
                nc.sync.dma_start(ysb[64:64 + _K, off:off + w],
                                  y_d[:, off:off + w])
                off += w

            NCQ = _NA + (_ND + 1) // 2
            sall = cpool.tile([128, NCQ * _QT], f32)

            # PSUM: A units double-buffered at [0:1536],[1536:3072];
            # D units at [3072:3584],[3584:4096]
            ps = ppool.tile([128, 8 * 512], f32)

            rg_par = [0]

            def score_mm(qt, dst, t0, width):
                for j in range(width // 512):
                    rg = 64 * (rg_par[0] & 1)
                    rg_par[0] += 1
                    nc.tensor.matmul(
                        out=ps[:, dst + j * 512: dst + (j + 1) * 512],
                        lhsT=xsb[rg:rg + _K, qt * 128:(qt + 1) * 128],
                        rhs=ysb[rg:rg + _K, t0 + j * 512: t0 + (j + 1) * 512],
                        start=True, stop=True,
                        tile_position=(rg, 0),
                    )

            pcol = [0] * _QT
            gen = {'A': 0, 'D': 0}
            cur = [0] * _QT
            half = [None] * _QT       # pending first half of a D pair
            for typ in seq:
                for qt in range(_QT):
                    t0 = cur[qt]
                    if typ == 'A':
                        dst = 1536 * (gen['A'] & 1)
                        gen['A'] += 1
                        score_mm(qt, dst, t0, 1536)
                        c = qt * NCQ + pcol[qt]
                        pcol[qt] += 1
                        nc.scalar.activation(
                            ps[:, dst:dst + 1536], ps[:, dst:dst + 1536],
                            mybir.ActivationFunctionType.Exp,
                            bias=bias_sb[:], scale=1.0 / _C1,
                            accum_out=sall[:, c:c + 1])
                        cur[qt] = t0 + 1536
                    else:
                        dst = 3072 + 512 * (gen['D'] & 1)
                        gen['D'] += 1
                        score_mm(qt, dst, t0, 512)
                        if half[qt] is None:
                            q16 = qpool.tile([128, 2, 512], i16)
                            nc.vector.tensor_scalar(
                                q16[:, 0, :], ps[:, dst:dst + 512], 32.0, 0.0,
                                mybir.AluOpType.mult, mybir.AluOpType.max)
                            half[qt] = q16
                        else:
                            q16 = half[qt]
                            half[qt] = None
                            nc.vector.tensor_scalar(
                                q16[:, 1, :], ps[:, dst:dst + 512], 32.0, 0.0,
                                mybir.AluOpType.mult, mybir.AluOpType.max)
                            c = qt * NCQ + pcol[qt]
                            pcol[qt] += 1
                            nc.vector.tensor_reduce(
                                sall[:, c:c + 1], q16[:].bitcast(bf16),
                                axis=mybir.AxisListType.XY,
                                op=mybir.AluOpType.add)
                        cur[qt] = t0 + 512

            for qt in range(_QT):
                if half[qt] is not None:
                    q16 = half[qt]
                    c = qt * NCQ + pcol[qt]
                    pcol[qt] += 1
                    nc.vector.tensor_reduce(
                        sall[:, c:c + 1], q16[:, 0, :].bitcast(bf16),
                        axis=mybir.AxisListType.X, op=mybir.AluOpType.add)

            fin = spool.tile([128, _QT], f32)
            for qt in range(_QT):
                nc.vector.tensor_reduce(
                    fin[:, qt:qt + 1], sall[:, qt * NCQ:qt * NCQ + pcol[qt]],
                    axis=mybir.AxisListType.X, op=mybir.AluOpType.add)
            nc.sync.dma_start(out_d[:], fin[:])

    nc.compile()
    return nc


def _get_program(n_trains: int):
    if n_trains not in _prog_cache:
        _prog_cache[n_trains] = _build_program(n_trains)
    return _prog_cache[n_trains]


def _prep_inputs(X, X_train, sample_weight):
    X = np.ascontiguousarray(np.asarray(X, dtype=np.float32))
    Y = np.ascontiguousarray(np.asarray(X_train, dtype=np.float32))
    w = np.ascontiguousarray(np.asarray(sample_weight, dtype=np.float32))
    n = Y.shape[0]

    w64 = w.astype(np.float64)
    b64 = np.log(np.maximum(w64, 1e-300)) - 0.5 * np.sum(
        Y.astype(np.float64) ** 2, axis=1)
    b64 = np.clip(b64, -35.0, None)
    cb64 = (_C1 * b64 + _C2B / 32.0) / 4.0
    bhi = cb64.astype(np.float32).astype(_BF16)
    blo = (cb64 - bhi.astype(np.float64)).astype(np.float32).astype(_BF16)

    yext = np.empty((_K, n), dtype=_BF16)
    yext[0:32] = Y.astype(_BF16).T
    yext[32] = bhi
    yext[33] = blo

    const = 0.5 * _D * np.log(2.0 * np.pi) + np.log(np.sum(w64))
    xsq = np.sum(X.astype(np.float64) ** 2, axis=1)
    dv_all = (0.5 * xsq + const).astype(np.float32)

    in_maps = []
    dvs = []
    for c in range(_NCORES):
        sl = slice(c * _QSHARD, (c + 1) * _QSHARD)
        xq = X[sl]
        xext = np.empty((_K, _QSHARD), dtype=_BF16)
        xext[0:32] = (_C1 * xq.astype(np.float64)).astype(_BF16).T
        xext[32] = np.full(_QSHARD, 4.0, dtype=_BF16)
        xext[33] = np.full(_QSHARD, 4.0, dtype=_BF16)
        dv = np.ascontiguousarray(dv_all[sl].reshape(_QT, 128).T)
        in_maps.append({"yext": yext, "xext": xext})
        dvs.append(dv_all[sl].astype(np.float64))
    return in_maps, dvs


def _gather(results, dvs):
    out = np.empty(_Q, dtype=np.float32)
    for c in range(_NCORES):
        tot = results[c]["out"].T.reshape(_QSHARD).astype(np.float64)
        out[c * _QSHARD:(c + 1) * _QSHARD] = np.log(tot) - dvs[c]
    return out


def kernel(X, X_train, sample_weight, _want_timing=False):
    from concourse.bass_utils import run_bass_kernel_spmd

    nc = _get_program(_N)
    in_maps, dvs = _prep_inputs(X, X_train, sample_weight)
    kres = run_bass_kernel_spmd(
        nc, in_maps, core_ids=list(range(_NCORES)),
        trace=bool(_want_timing),
    )
    out = _gather(kres.results, dvs)
    if _want_timing:
        return out, kres
    return out


# revision 27
# speedup vs baseline: 2.4173x; 1.0134x over previous
"""Gaussian KDE (brute-force, bandwidth^2 = 1) on 8 Trainium2 NeuronCores.

Math:
    out_i = log( sum_j w_j * exp(-||x_i - y_j||^2/2) ) - (d/2) log(2pi) - log(sum_j w_j)
          = log( sum_j exp(x_i . y_j + b_j) ) - ||x_i||^2/2 - consts
    with b_j = log(w_j) - ||y_j||^2/2.

Queries sharded 8 ways (512/core, 4 PSUM-partition tiles). Per core:
    - scores: K=35 bf16 matmuls, stationary = query tile [35, 128], moving =
      train slices. Operands are pre-scaled so PSUM holds C1*s + C2b/32
      (C1 = 4/ln2, C2b the bf16 Schraudolph bias): x rows = C1*x dims plus
      three 4.0 rows; y rows = y dims + (C1/4)*b hi + lo + C2b/128 row.
      K=35 <= 64, so consecutive matmuls alternate PE row groups via
      tile_position (0,0)/(64,0) and run pairwise-concurrently.
    - exp+sum, two unit types balanced by a credit scheduler across engines:
      * ACT unit [128, 1536] (3 banks): table-exp in place with
        scale=1/C1, bias=-C2b/(32*C1), free-dim sum fused via accum_out.
      * DVE unit [128, 512] (1 bank): tensor_scalar (mult 32, max 0) whose
        int16 result IS the bf16 bit pattern of exp (Schraudolph; the max
        makes int16 wrap impossible), then tensor_reduce of the bitcast.
    - final: per query tile reduce partials, ln, subtract per-query const.
"""

import numpy as np
import ml_dtypes

_Q, _N, _D = 4096, 65536, 32
_NCORES = 8
_QSHARD = _Q // _NCORES          # 512 queries per core
_K = 34                          # 32 dims + bias hi/lo (incl C2 const)
_QT = 4                          # query tiles per core

_BF16 = ml_dtypes.bfloat16

_C1 = 4.0 / float(np.log(2.0))


def _c2b():
    f = (np.arange(100000, dtype=np.float64) + 0.5) / 100000.0
    m0 = np.mean((1.0 + f) * 2.0 ** (-f))
    m1 = np.mean(2.0 ** (-f))
    delta = (m0 - 1.0) / m1
    return float(127 * 128 - delta * 128)


_C2B = _c2b()

# per query tile: trains covered by ACT units (1536 each) and DVE units (512)
_NA = 30                         # 30 * 1536 = 46080
_ND = 38                         # 38 * 512  = 19456 ; total 65536
_ACT_NS = 1660.0                 # measured per ACT unit
_DVE_NS = 1294.0                 # measured per DVE unit (incl half reduce)

_prog_cache: dict = {}


def _unit_schedule():
    """Credit-scheduled unit type sequence (shared by all 4 query tiles)."""
    seq = []
    na, nd, ta, td = 0, 0, 0.0, -5000.0   # start with D units: less y needed
    while na < _NA or nd < _ND:
        if nd >= _ND or (na < _NA and ta <= td):
            seq.append('A')
            na += 1
            ta += _ACT_NS
        else:
            seq.append('D')
            nd += 1
            td += _DVE_NS
    return seq


def _build_program(n_trains: int):
    import concourse.bass as bass
    import concourse.tile as tile
    from concourse import bacc, mybir

    f32 = mybir.dt.float32
    bf16 = mybir.dt.bfloat16
    i16 = mybir.dt.int16

    nc = bacc.Bacc("TRN2", target_bir_lowering=False, debug=False,
                   num_devices=_NCORES)

    y_d = nc.dram_tensor("yext", [_K, n_trains], bf16, kind="ExternalInput")
    x_d = nc.dram_tensor("xext", [_K, _QSHARD], bf16, kind="ExternalInput")
    out_d = nc.dram_tensor("out", [128, _QT], f32, kind="ExternalOutput")

    seq = _unit_schedule()

    with tile.TileContext(nc) as tc:
        with (
            tc.tile_pool(name="const", bufs=1) as cpool,
            tc.tile_pool(name="q16", bufs=8) as qpool,
            tc.tile_pool(name="small", bufs=2) as spool,
            tc.tile_pool(name="ps", bufs=1, space="PSUM") as ppool,
        ):
            xsb = cpool.tile([128, _QSHARD], bf16)
            nc.sync.dma_start(xsb[0:_K, :], x_d[:])
            nc.sync.dma_start(xsb[64:64 + _K, :], x_d[:])
            bias_sb = cpool.tile([128, 1], f32)
            nc.vector.memset(bias_sb[:], -_C2B / (32.0 * _C1))

            # y resident in SBUF, both row-group strips, 8 DMA pieces each
            ysb = cpool.tile([128, n_trains], bf16)
            pieces = [1024, 1024, 2048, 4096] + [8192] * 7
            off = 0
            for w in pieces:
                nc.sync.dma_start(ysb[0:_K, off:off + w],
                                  y_d[:, off:off + w])
                nc.sync.dma_start(ysb[64:64 + _K, off:off + w],
                                  y_d[:, off:off + w])
                off += w

            NCQ = _NA + (_ND + 1) // 2
            sall = cpool.tile([128, NCQ * _QT], f32)

            # PSUM: A units double-buffered at [0:1536],[1536:3072];
            # D units at [3072:3584],[3584:4096]
            ps = ppool.tile([128, 8 * 512], f32)

            rg_par = [0]

            def score_mm(qt, dst, t0, width):
                for j in range(width // 512):
                    rg = 64 * (rg_par[0] & 1)
                    rg_par[0] += 1
                    nc.tensor.matmul(
                        out=ps[:, dst + j * 512: dst + (j + 1) * 512],
                        lhsT=xsb[rg:rg + _K, qt * 128:(qt + 1) * 128],
                        rhs=ysb[rg:rg + _K, t0 + j * 512: t0 + (j + 1) * 512],
                        start=True, stop=True,
                        tile_position=(rg, 0),
                    )

            pcol = [0] * _QT
            gen = {'A': 0, 'D': 0}
            cur = [0] * _QT
            half = [None] * _QT       # pending first half of a D pair
            for typ in seq:
                for qt in range(_QT):
                    t0 = cur[qt]
                    if typ == 'A':
                        dst = 1536 * (gen['A'] & 1)
                        gen['A'] += 1
                        score_mm(qt, dst, t0, 1536)
                        c = qt * NCQ + pcol[qt]
                        pcol[qt] += 1
                        nc.scalar.activation(
                            ps[:, dst:dst + 1536], ps[:, dst:dst + 1536],
                            mybir.ActivationFunctionType.Exp,
                            bias=bias_sb[:], scale=1.0 / _C1,
                            accum_out=sall[:, c:c + 1])
                        cur[qt] = t0 + 1536
                    else:
                        dst = 3072 + 512 * (gen['D'] & 1)
                        gen['D'] += 1
                        score_mm(qt, dst, t0, 512)
                        if half[qt] is None:
                            q16 = qpool.tile([128, 2, 512], i16)
                            nc.vector.tensor_scalar(
                                q16[:, 0, :], ps[:, dst:dst + 512], 32.0, 0.0,
                                mybir.AluOpType.mult, mybir.AluOpType.max)
                            half[qt] = q16
                        else:
                            q16 = half[qt]
                            half[qt] = None
                            nc.vector.tensor_scalar(
                                q16[:, 1, :], ps[:, dst:dst + 512], 32.0, 0.0,
                                mybir.AluOpType.mult, mybir.AluOpType.max)
                            c = qt * NCQ + pcol[qt]
                            pcol[qt] += 1
                            nc.vector.tensor_reduce(
                                sall[:, c:c + 1], q16[:].bitcast(bf16),
                                axis=mybir.AxisListType.XY,
                                op=mybir.AluOpType.add)
                        cur[qt] = t0 + 512

            for qt in range(_QT):
                if half[qt] is not None:
                    q16 = half[qt]
                    c = qt * NCQ + pcol[qt]
                    pcol[qt] += 1
                    nc.vector.tensor_reduce(
                        sall[:, c:c + 1], q16[:, 0, :].bitcast(bf16),
                        axis=mybir.AxisListType.X, op=mybir.AluOpType.add)

            fin = spool.tile([128, _QT], f32)
            for qt in range(_QT):
                nc.vector.tensor_reduce(
                    fin[:, qt:qt + 1], sall[:, qt * NCQ:qt * NCQ + pcol[qt]],
                    axis=mybir.AxisListType.X, op=mybir.AluOpType.add)
            nc.sync.dma_start(out_d[:], fin[:])

    nc.compile()
    return nc


def _get_program(n_trains: int):
    if n_trains not in _prog_cache:
        _prog_cache[n_trains] = _build_program(n_trains)
    return _prog_cache[n_trains]


def _prep_inputs(X, X_train, sample_weight):
    X = np.ascontiguousarray(np.asarray(X, dtype=np.float32))
    Y = np.ascontiguousarray(np.asarray(X_train, dtype=np.float32))
    w = np.ascontiguousarray(np.asarray(sample_weight, dtype=np.float32))
    n = Y.shape[0]

    w64 = w.astype(np.float64)
    b64 = np.log(np.maximum(w64, 1e-300)) - 0.5 * np.sum(
        Y.astype(np.float64) ** 2, axis=1)
    b64 = np.clip(b64, -35.0, None)
    cb64 = (_C1 * b64 + _C2B / 32.0) / 4.0
    bhi = cb64.astype(np.float32).astype(_BF16)
    blo = (cb64 - bhi.astype(np.float64)).astype(np.float32).astype(_BF16)

    yext = np.empty((_K, n), dtype=_BF16)
    yext[0:32] = Y.astype(_BF16).T
    yext[32] = bhi
    yext[33] = blo

    const = 0.5 * _D * np.log(2.0 * np.pi) + np.log(np.sum(w64))
    xsq = np.sum(X.astype(np.float64) ** 2, axis=1)
    dv_all = (0.5 * xsq + const).astype(np.float32)

    in_maps = []
    dvs = []
    for c in range(_NCORES):
        sl = slice(c * _QSHARD, (c + 1) * _QSHARD)
        xq = X[sl]
        xext = np.empty((_K, _QSHARD), dtype=_BF16)
        xext[0:32] = (_C1 * xq.astype(np.float64)).astype(_BF16).T
        xext[32] = np.full(_QSHARD, 4.0, dtype=_BF16)
        xext[33] = np.full(_QSHARD, 4.0, dtype=_BF16)
        dv = np.ascontiguousarray(dv_all[sl].reshape(_QT, 128).T)
        in_maps.append({"yext": yext, "xext": xext})
        dvs.append(dv_all[sl].astype(np.float64))
    return in_maps, dvs


def _gather(results, dvs):
    out = np.empty(_Q, dtype=np.float32)
    for c in range(_NCORES):
        tot = results[c]["out"].T.reshape(_QSHARD).astype(np.float64)
        out[c * _QSHARD:(c + 1) * _QSHARD] = np.log(tot) - dvs[c]
    return out


def kernel(X, X_train, sample_weight, _want_timing=False):
    from concourse.bass_utils import run_bass_kernel_spmd

    nc = _get_program(_N)
    in_maps, dvs = _prep_inputs(X, X_train, sample_weight)
    kres = run_bass_kernel_spmd(
        nc, in_maps, core_ids=list(range(_NCORES)),
        trace=bool(_want_timing),
    )
    out = _gather(kres.results, dvs)
    if _want_timing:
        return out, kres
    return out


# revision 29
# speedup vs baseline: 2.5057x; 1.0366x over previous
"""Gaussian KDE (brute-force, bandwidth^2 = 1) on 8 Trainium2 NeuronCores.

Math:
    out_i = log( sum_j w_j * exp(-||x_i - y_j||^2/2) ) - (d/2) log(2pi) - log(sum_j w_j)
          = log( sum_j exp(x_i . y_j + b_j) ) - ||x_i||^2/2 - consts
    with b_j = log(w_j) - ||y_j||^2/2.

Queries sharded 8 ways (512/core, 4 PSUM-partition tiles). Per core:
    - scores: K=35 bf16 matmuls, stationary = query tile [35, 128], moving =
      train slices. Operands are pre-scaled so PSUM holds C1*s + C2b/32
      (C1 = 4/ln2, C2b the bf16 Schraudolph bias): x rows = C1*x dims plus
      three 4.0 rows; y rows = y dims + (C1/4)*b hi + lo + C2b/128 row.
      K=35 <= 64, so consecutive matmuls alternate PE row groups via
      tile_position (0,0)/(64,0) and run pairwise-concurrently.
    - exp+sum, two unit types balanced by a credit scheduler across engines:
      * ACT unit [128, 1536] (3 banks): table-exp in place with
        scale=1/C1, bias=-C2b/(32*C1), free-dim sum fused via accum_out.
      * DVE unit [128, 512] (1 bank): tensor_scalar (mult 32, max 0) whose
        int16 result IS the bf16 bit pattern of exp (Schraudolph; the max
        makes int16 wrap impossible), then tensor_reduce of the bitcast.
    - final: per query tile reduce partials, ln, subtract per-query const.
"""

import numpy as np
import ml_dtypes

_Q, _N, _D = 4096, 65536, 32
_NCORES = 8
_QSHARD = _Q // _NCORES          # 512 queries per core
_K = 34                          # 32 dims + bias hi/lo (incl C2 const)
_QT = 4                          # query tiles per core

_BF16 = ml_dtypes.bfloat16

_C1 = 4.0 / float(np.log(2.0))


def _c2b():
    f = (np.arange(100000, dtype=np.float64) + 0.5) / 100000.0
    m0 = np.mean((1.0 + f) * 2.0 ** (-f))
    m1 = np.mean(2.0 ** (-f))
    delta = (m0 - 1.0) / m1
    return float(127 * 128 - delta * 128)


_C2B = _c2b()

# per query tile: trains covered by ACT units (1536 each) and DVE units (512)
_NA = 30                         # 30 * 1536 = 46080
_ND = 38                         # 38 * 512  = 19456 ; total 65536
_ACT_NS = 1660.0                 # measured per ACT unit
_DVE_NS = 1294.0                 # measured per DVE unit (incl half reduce)

_prog_cache: dict = {}


def _unit_schedule():
    """Credit-scheduled unit type sequence (shared by all 4 query tiles)."""
    seq = []
    na, nd, ta, td = 0, 0, 0.0, -5000.0   # start with D units: less y needed
    while na < _NA or nd < _ND:
        if nd >= _ND or (na < _NA and ta <= td):
            seq.append('A')
            na += 1
            ta += _ACT_NS
        else:
            seq.append('D')
            nd += 1
            td += _DVE_NS
    return seq


def _build_program(n_trains: int):
    import concourse.bass as bass
    import concourse.tile as tile
    from concourse import bacc, mybir

    f32 = mybir.dt.float32
    bf16 = mybir.dt.bfloat16
    i16 = mybir.dt.int16

    nc = bacc.Bacc("TRN2", target_bir_lowering=False, debug=False,
                   num_devices=_NCORES)

    y_d = nc.dram_tensor("yext", [_K, n_trains], bf16, kind="ExternalInput")
    x_d = nc.dram_tensor("xext", [_K, _QSHARD], bf16, kind="ExternalInput")
    out_d = nc.dram_tensor("out", [128, _QT], f32, kind="ExternalOutput")

    seq = _unit_schedule()

    with tile.TileContext(nc) as tc:
        with (
            tc.tile_pool(name="const", bufs=1) as cpool,
            tc.tile_pool(name="q16", bufs=8) as qpool,
            tc.tile_pool(name="small", bufs=2) as spool,
            tc.tile_pool(name="ps", bufs=1, space="PSUM") as ppool,
        ):
            xsb = cpool.tile([128, _QSHARD], bf16)
            nc.sync.dma_start(xsb[0:_K, :], x_d[:])
            nc.sync.dma_start(xsb[64:64 + _K, :], x_d[:])
            bias_sb = cpool.tile([128, 1], f32)
            nc.vector.memset(bias_sb[:], -_C2B / (32.0 * _C1))

            # y resident in SBUF, both row-group strips, 8 DMA pieces each
            ysb = cpool.tile([128, n_trains], bf16)
            pieces = [1024, 1024, 2048] + [4096] * 15
            off = 0
            for w in pieces:
                nc.sync.dma_start(ysb[0:_K, off:off + w],
                                  y_d[:, off:off + w])
                nc.sync.dma_start(ysb[64:64 + _K, off:off + w],
                                  y_d[:, off:off + w])
                off += w

            NCQ = _NA + (_ND + 1) // 2
            sall = cpool.tile([128, NCQ * _QT], f32)

            # PSUM: A units double-buffered at [0:1536],[1536:3072];
            # D units at [3072:3584],[3584:4096]
            ps = ppool.tile([128, 8 * 512], f32)

            rg_par = [0]

            def score_mm(qt, dst, t0, width):
                for j in range(width // 512):
                    rg = 64 * (rg_par[0] & 1)
                    rg_par[0] += 1
                    nc.tensor.matmul(
                        out=ps[:, dst + j * 512: dst + (j + 1) * 512],
                        lhsT=xsb[rg:rg + _K, qt * 128:(qt + 1) * 128],
                        rhs=ysb[rg:rg + _K, t0 + j * 512: t0 + (j + 1) * 512],
                        start=True, stop=True,
                        tile_position=(rg, 0),
                    )

            pcol = [0] * _QT
            gen = {'A': 0, 'D': 0}
            cur = [0] * _QT
            half = [None] * _QT       # pending first half of a D pair
            for typ in seq:
                for qt in range(_QT):
                    t0 = cur[qt]
                    if typ == 'A':
                        dst = 1536 * (gen['A'] & 1)
                        gen['A'] += 1
                        score_mm(qt, dst, t0, 1536)
                        c = qt * NCQ + pcol[qt]
                        pcol[qt] += 1
                        nc.scalar.activation(
                            ps[:, dst:dst + 1536], ps[:, dst:dst + 1536],
                            mybir.ActivationFunctionType.Exp,
                            bias=bias_sb[:], scale=1.0 / _C1,
                            accum_out=sall[:, c:c + 1])
                        cur[qt] = t0 + 1536
                    else:
                        dst = 3072 + 512 * (gen['D'] & 1)
                        gen['D'] += 1
                        score_mm(qt, dst, t0, 512)
                        if half[qt] is None:
                            q16 = qpool.tile([128, 2, 512], i16)
                            nc.vector.tensor_scalar(
                                q16[:, 0, :], ps[:, dst:dst + 512], 32.0, 0.0,
                                mybir.AluOpType.mult, mybir.AluOpType.max)
                            half[qt] = q16
                        else:
                            q16 = half[qt]
                            half[qt] = None
                            nc.vector.tensor_scalar(
                                q16[:, 1, :], ps[:, dst:dst + 512], 32.0, 0.0,
                                mybir.AluOpType.mult, mybir.AluOpType.max)
                            c = qt * NCQ + pcol[qt]
                            pcol[qt] += 1
                            nc.vector.tensor_reduce(
                                sall[:, c:c + 1], q16[:].bitcast(bf16),
                                axis=mybir.AxisListType.XY,
                                op=mybir.AluOpType.add)
                        cur[qt] = t0 + 512

            for qt in range(_QT):
                if half[qt] is not None:
                    q16 = half[qt]
                    c = qt * NCQ + pcol[qt]
                    pcol[qt] += 1
                    nc.vector.tensor_reduce(
                        sall[:, c:c + 1], q16[:, 0, :].bitcast(bf16),
                        axis=mybir.AxisListType.X, op=mybir.AluOpType.add)

            fin = spool.tile([128, _QT], f32)
            for qt in range(_QT):
                nc.vector.tensor_reduce(
                    fin[:, qt:qt + 1], sall[:, qt * NCQ:qt * NCQ + pcol[qt]],
                    axis=mybir.AxisListType.X, op=mybir.AluOpType.add)
            nc.sync.dma_start(out_d[:], fin[:])

    nc.compile()
    return nc


def _get_program(n_trains: int):
    if n_trains not in _prog_cache:
        _prog_cache[n_trains] = _build_program(n_trains)
    return _prog_cache[n_trains]


def _prep_inputs(X, X_train, sample_weight):
    X = np.ascontiguousarray(np.asarray(X, dtype=np.float32))
    Y = np.ascontiguousarray(np.asarray(X_train, dtype=np.float32))
    w = np.ascontiguousarray(np.asarray(sample_weight, dtype=np.float32))
    n = Y.shape[0]

    w64 = w.astype(np.float64)
    b64 = np.log(np.maximum(w64, 1e-300)) - 0.5 * np.sum(
        Y.astype(np.float64) ** 2, axis=1)
    b64 = np.clip(b64, -35.0, None)
    cb64 = (_C1 * b64 + _C2B / 32.0) / 4.0
    bhi = cb64.astype(np.float32).astype(_BF16)
    blo = (cb64 - bhi.astype(np.float64)).astype(np.float32).astype(_BF16)

    yext = np.empty((_K, n), dtype=_BF16)
    yext[0:32] = Y.astype(_BF16).T
    yext[32] = bhi
    yext[33] = blo

    const = 0.5 * _D * np.log(2.0 * np.pi) + np.log(np.sum(w64))
    xsq = np.sum(X.astype(np.float64) ** 2, axis=1)
    dv_all = (0.5 * xsq + const).astype(np.float32)

    in_maps = []
    dvs = []
    for c in range(_NCORES):
        sl = slice(c * _QSHARD, (c + 1) * _QSHARD)
        xq = X[sl]
        xext = np.empty((_K, _QSHARD), dtype=_BF16)
        xext[0:32] = (_C1 * xq.astype(np.float64)).astype(_BF16).T
        xext[32] = np.full(_QSHARD, 4.0, dtype=_BF16)
        xext[33] = np.full(_QSHARD, 4.0, dtype=_BF16)
        dv = np.ascontiguousarray(dv_all[sl].reshape(_QT, 128).T)
        in_maps.append({"yext": yext, "xext": xext})
        dvs.append(dv_all[sl].astype(np.float64))
    return in_maps, dvs


def _gather(results, dvs):
    out = np.empty(_Q, dtype=np.float32)
    for c in range(_NCORES):
        tot = results[c]["out"].T.reshape(_QSHARD).astype(np.float64)
        out[c * _QSHARD:(c + 1) * _QSHARD] = np.log(tot) - dvs[c]
    return out


def kernel(X, X_train, sample_weight, _want_timing=False):
    from concourse.bass_utils import run_bass_kernel_spmd

    nc = _get_program(_N)
    in_maps, dvs = _prep_inputs(X, X_train, sample_weight)
    kres = run_bass_kernel_spmd(
        nc, in_maps, core_ids=list(range(_NCORES)),
        trace=bool(_want_timing),
    )
    out = _gather(kres.results, dvs)
    if _want_timing:
        return out, kres
    return out


# revision 30
# speedup vs baseline: 2.5673x; 1.0246x over previous
"""Gaussian KDE (brute-force, bandwidth^2 = 1) on 8 Trainium2 NeuronCores.

Math:
    out_i = log( sum_j w_j * exp(-||x_i - y_j||^2/2) ) - (d/2) log(2pi) - log(sum_j w_j)
          = log( sum_j exp(x_i . y_j + b_j) ) - ||x_i||^2/2 - consts
    with b_j = log(w_j) - ||y_j||^2/2.

Queries sharded 8 ways (512/core, 4 PSUM-partition tiles). Per core:
    - scores: K=35 bf16 matmuls, stationary = query tile [35, 128], moving =
      train slices. Operands are pre-scaled so PSUM holds C1*s + C2b/32
      (C1 = 4/ln2, C2b the bf16 Schraudolph bias): x rows = C1*x dims plus
      three 4.0 rows; y rows = y dims + (C1/4)*b hi + lo + C2b/128 row.
      K=35 <= 64, so consecutive matmuls alternate PE row groups via
      tile_position (0,0)/(64,0) and run pairwise-concurrently.
    - exp+sum, two unit types balanced by a credit scheduler across engines:
      * ACT unit [128, 1536] (3 banks): table-exp in place with
        scale=1/C1, bias=-C2b/(32*C1), free-dim sum fused via accum_out.
      * DVE unit [128, 512] (1 bank): tensor_scalar (mult 32, max 0) whose
        int16 result IS the bf16 bit pattern of exp (Schraudolph; the max
        makes int16 wrap impossible), then tensor_reduce of the bitcast.
    - final: per query tile reduce partials, ln, subtract per-query const.
"""

import numpy as np
import ml_dtypes

_Q, _N, _D = 4096, 65536, 32
_NCORES = 8
_QSHARD = _Q // _NCORES          # 512 queries per core
_K = 34                          # 32 dims + bias hi/lo (incl C2 const)
_QT = 4                          # query tiles per core

_BF16 = ml_dtypes.bfloat16

_C1 = 4.0 / float(np.log(2.0))


def _c2b():
    f = (np.arange(100000, dtype=np.float64) + 0.5) / 100000.0
    m0 = np.mean((1.0 + f) * 2.0 ** (-f))
    m1 = np.mean(2.0 ** (-f))
    delta = (m0 - 1.0) / m1
    return float(127 * 128 - delta * 128)


_C2B = _c2b()

# per query tile: trains covered by ACT units (1536 each) and DVE units (512)
_NA = 30                         # 30 * 1536 = 46080
_ND = 38                         # 38 * 512  = 19456 ; total 65536
_ACT_NS = 1660.0                 # measured per ACT unit
_DVE_NS = 1294.0                 # measured per DVE unit (incl half reduce)

_prog_cache: dict = {}


def _unit_schedule():
    """Credit-scheduled unit type sequence (shared by all 4 query tiles)."""
    seq = []
    na, nd, ta, td = 0, 0, 0.0, -5000.0   # start with D units: less y needed
    while na < _NA or nd < _ND:
        if nd >= _ND or (na < _NA and ta <= td):
            seq.append('A')
            na += 1
            ta += _ACT_NS
        else:
            seq.append('D')
            nd += 1
            td += _DVE_NS
    return seq


def _build_program(n_trains: int):
    import concourse.bass as bass
    import concourse.tile as tile
    from concourse import bacc, mybir

    f32 = mybir.dt.float32
    bf16 = mybir.dt.bfloat16
    i16 = mybir.dt.int16

    nc = bacc.Bacc("TRN2", target_bir_lowering=False, debug=False,
                   num_devices=_NCORES)

    y_d = nc.dram_tensor("yext", [_K, n_trains], bf16, kind="ExternalInput")
    x_d = nc.dram_tensor("xext", [_K, _QSHARD], bf16, kind="ExternalInput")
    out_d = nc.dram_tensor("out", [128, _QT], f32, kind="ExternalOutput")

    seq = _unit_schedule()

    with tile.TileContext(nc) as tc:
        with (
            tc.tile_pool(name="const", bufs=1) as cpool,
            tc.tile_pool(name="q16", bufs=8) as qpool,
            tc.tile_pool(name="small", bufs=2) as spool,
            tc.tile_pool(name="ps", bufs=1, space="PSUM") as ppool,
        ):
            xsb = cpool.tile([128, _QSHARD], bf16)
            nc.sync.dma_start(xsb[0:_K, :], x_d[:])
            nc.sync.dma_start(xsb[64:64 + _K, :], x_d[:])
            bias_sb = cpool.tile([128, 1], f32)
            nc.vector.memset(bias_sb[:], -_C2B / (32.0 * _C1))

            # y resident in SBUF, both row-group strips, 8 DMA pieces each
            ysb = cpool.tile([128, n_trains], bf16)
            pieces = [1024, 1024] + [2048] * 31
            off = 0
            for w in pieces:
                nc.sync.dma_start(ysb[0:_K, off:off + w],
                                  y_d[:, off:off + w])
                nc.sync.dma_start(ysb[64:64 + _K, off:off + w],
                                  y_d[:, off:off + w])
                off += w

            NCQ = _NA + (_ND + 1) // 2
            sall = cpool.tile([128, NCQ * _QT], f32)

            # PSUM: A units double-buffered at [0:1536],[1536:3072];
            # D units at [3072:3584],[3584:4096]
            ps = ppool.tile([128, 8 * 512], f32)

            rg_par = [0]

            def score_mm(qt, dst, t0, width):
                for j in range(width // 512):
                    rg = 64 * (rg_par[0] & 1)
                    rg_par[0] += 1
                    nc.tensor.matmul(
                        out=ps[:, dst + j * 512: dst + (j + 1) * 512],
                        lhsT=xsb[rg:rg + _K, qt * 128:(qt + 1) * 128],
                        rhs=ysb[rg:rg + _K, t0 + j * 512: t0 + (j + 1) * 512],
                        start=True, stop=True,
                        tile_position=(rg, 0),
                    )

            pcol = [0] * _QT
            gen = {'A': 0, 'D': 0}
            cur = [0] * _QT
            half = [None] * _QT       # pending first half of a D pair
            for typ in seq:
                for qt in range(_QT):
                    t0 = cur[qt]
                    if typ == 'A':
                        dst = 1536 * (gen['A'] & 1)
                        gen['A'] += 1
                        score_mm(qt, dst, t0, 1536)
                        c = qt * NCQ + pcol[qt]
                        pcol[qt] += 1
                        nc.scalar.activation(
                            ps[:, dst:dst + 1536], ps[:, dst:dst + 1536],
                            mybir.ActivationFunctionType.Exp,
                            bias=bias_sb[:], scale=1.0 / _C1,
                            accum_out=sall[:, c:c + 1])
                        cur[qt] = t0 + 1536
                    else:
                        dst = 3072 + 512 * (gen['D'] & 1)
                        gen['D'] += 1
                        score_mm(qt, dst, t0, 512)
                        if half[qt] is None:
                            q16 = qpool.tile([128, 2, 512], i16)
                            nc.vector.tensor_scalar(
                                q16[:, 0, :], ps[:, dst:dst + 512], 32.0, 0.0,
                                mybir.AluOpType.mult, mybir.AluOpType.max)
                            half[qt] = q16
                        else:
                            q16 = half[qt]
                            half[qt] = None
                            nc.vector.tensor_scalar(
                                q16[:, 1, :], ps[:, dst:dst + 512], 32.0, 0.0,
                                mybir.AluOpType.mult, mybir.AluOpType.max)
                            c = qt * NCQ + pcol[qt]
                            pcol[qt] += 1
                            nc.vector.tensor_reduce(
                                sall[:, c:c + 1], q16[:].bitcast(bf16),
                                axis=mybir.AxisListType.XY,
                                op=mybir.AluOpType.add)
                        cur[qt] = t0 + 512

            for qt in range(_QT):
                if half[qt] is not None:
                    q16 = half[qt]
                    c = qt * NCQ + pcol[qt]
                    pcol[qt] += 1
                    nc.vector.tensor_reduce(
                        sall[:, c:c + 1], q16[:, 0, :].bitcast(bf16),
                        axis=mybir.AxisListType.X, op=mybir.AluOpType.add)

            fin = spool.tile([128, _QT], f32)
            for qt in range(_QT):
                nc.vector.tensor_reduce(
                    fin[:, qt:qt + 1], sall[:, qt * NCQ:qt * NCQ + pcol[qt]],
                    axis=mybir.AxisListType.X, op=mybir.AluOpType.add)
            nc.sync.dma_start(out_d[:], fin[:])

    nc.compile()
    return nc


def _get_program(n_trains: int):
    if n_trains not in _prog_cache:
        _prog_cache[n_trains] = _build_program(n_trains)
    return _prog_cache[n_trains]


def _prep_inputs(X, X_train, sample_weight):
    X = np.ascontiguousarray(np.asarray(X, dtype=np.float32))
    Y = np.ascontiguousarray(np.asarray(X_train, dtype=np.float32))
    w = np.ascontiguousarray(np.asarray(sample_weight, dtype=np.float32))
    n = Y.shape[0]

    w64 = w.astype(np.float64)
    b64 = np.log(np.maximum(w64, 1e-300)) - 0.5 * np.sum(
        Y.astype(np.float64) ** 2, axis=1)
    b64 = np.clip(b64, -35.0, None)
    cb64 = (_C1 * b64 + _C2B / 32.0) / 4.0
    bhi = cb64.astype(np.float32).astype(_BF16)
    blo = (cb64 - bhi.astype(np.float64)).astype(np.float32).astype(_BF16)

    yext = np.empty((_K, n), dtype=_BF16)
    yext[0:32] = Y.astype(_BF16).T
    yext[32] = bhi
    yext[33] = blo

    const = 0.5 * _D * np.log(2.0 * np.pi) + np.log(np.sum(w64))
    xsq = np.sum(X.astype(np.float64) ** 2, axis=1)
    dv_all = (0.5 * xsq + const).astype(np.float32)

    in_maps = []
    dvs = []
    for c in range(_NCORES):
        sl = slice(c * _QSHARD, (c + 1) * _QSHARD)
        xq = X[sl]
        xext = np.empty((_K, _QSHARD), dtype=_BF16)
        xext[0:32] = (_C1 * xq.astype(np.float64)).astype(_BF16).T
        xext[32] = np.full(_QSHARD, 4.0, dtype=_BF16)
        xext[33] = np.full(_QSHARD, 4.0, dtype=_BF16)
        dv = np.ascontiguousarray(dv_all[sl].reshape(_QT, 128).T)
        in_maps.append({"yext": yext, "xext": xext})
        dvs.append(dv_all[sl].astype(np.float64))
    return in_maps, dvs


def _gather(results, dvs):
    out = np.empty(_Q, dtype=np.float32)
    for c in range(_NCORES):
        tot = results[c]["out"].T.reshape(_QSHARD).astype(np.float64)
        out[c * _QSHARD:(c + 1) * _QSHARD] = np.log(tot) - dvs[c]
    return out


def kernel(X, X_train, sample_weight, _want_timing=False):
    from concourse.bass_utils import run_bass_kernel_spmd

    nc = _get_program(_N)
    in_maps, dvs = _prep_inputs(X, X_train, sample_weight)
    kres = run_bass_kernel_spmd(
        nc, in_maps, core_ids=list(range(_NCORES)),
        trace=bool(_want_timing),
    )
    out = _gather(kres.results, dvs)
    if _want_timing:
        return out, kres
    return out
